# revision 16
# baseline (speedup 1.0000x reference)
"""LinearQuant kernel for Trainium2 (8 NeuronCores, data parallel).

Reference math (fp32, bit-exact):
    delta = 2^-4; bound = 128
    out = clip(floor(x/delta + 0.5), -128, 127) * delta

Device pipeline (2 DVE tensor_scalar ops per tile, int8 output):

  r = RNE_int16(32*x + 0.5)          # TS mult+add, fp32 -> int16 (HW convert
                                     # is round-nearest-even + saturate;
                                     # |r| <= ~260 for any sane x)
  k = RNE_int8(r*0.5 - 0.25)         # TS mult+add, int16 -> int8: r even ->
                                     # k-0.25, r odd -> k+0.25, both RNE to
                                     # k = floor(16x+0.5) exactly (incl. ties,
                                     # negatives); int8 saturation == the
                                     # reference clamp to [-128, 127]

using the identity floor(u) = RNE(2u - 0.5) >> 1 with u = 16x + 0.5
(2u - 0.5 = 32x + 0.5), where the ">> 1" is done in float as *0.5 - 0.25
before the int8 convert. Verified bit-exact on HW for tie points, clamp
range, and 16K random normals (probe2.py). Host multiplies the int8 k by
2^-4 -- exact.

Storing int8 instead of bf16 halves store traffic: 25.7 MB in + 6.4 MB out
= 32.1 MB per core vs 38.5 MB for the bf16 baseline. Measured wire rate
while DMA-busy is ~410-415 GB/s (uncontended; the fabric ceiling is 435)
or ~355 GB/s when the paired NeuronCore / neighbors load the same HBM
stack, with zero mid-stream DMA gaps either way, so the kernel is at the
memory roofline: ~8.7 us fixed NEFF preamble + 32.1 MB wire time + ~3 us
tail latency (last in-DMA receipt -> p1 -> p2 -> out-DMA -> receipt ->
postamble). Measured 89.5-90.4 us uncontended, ~102-106 us contended
(baseline: 129-130 us).

Engine split: SP(sync) issues in-DMAs (HWDGE ring 1), DVE runs the two TS
passes software-pipelined (p1(i) || p2(i-1)) over ring buffers so no
same-engine commit stalls, ACT issues out-DMAs (HWDGE ring 2, no
activation => no ACT table load). Raw Block style with explicit
semaphores (Tile auto-sems hit walrus "Too many sync wait commands" on
this shape). Ring depths matter: DY=3 throttles out-DMA issue just
enough that write bursts stay coarse relative to the read stream --
DY=6 dropped the mixed read/write rate to ~343 GB/s (fine-grained HBM
read/write turnaround), DY=2 stalled p2 on out receipts, and issuing
everything on one HWDGE ring serialized in/out transfers (123 us).

Sharding: x(64,256,56,56) split 8-way along batch -> 6,422,528 elems/core
= 14 tiles of [128, 3584] fp32. The last tile is split into 4 quarter
chunks of [128, 896] so the post-last-in-DMA serial chain (p1+p2+out-DMA
+ DVE drain) shrinks from ~7 us to ~2 us.
"""

import os

import numpy as np

B, C, H, W = 64, 256, 56, 56
N_CORES = 8
P = 128          # partitions
F = 3584         # free elems per full tile
NT = 14          # full tiles per core:  8*256*56*56 == NT*P*F
DIN = 4          # input ring slots
DT = 3           # intermediate ring slots
DY = 3           # output ring slots
# chunk list: (tile_idx, col_lo, width); last tile tapered into quarters
CHUNKS = [(i, 0, F) for i in range(NT - 1)] + [
    (NT - 1, q * (F // 4), F // 4) for q in range(4)
]
NC_ = len(CHUNKS)

_cache = {}


def _build():
    from contextlib import ExitStack

    import concourse.mybir as mybir
    from concourse.bass import Bass

    fp32 = mybir.dt.float32
    int16 = mybir.dt.int16
    int8 = mybir.dt.int8
    alu = mybir.AluOpType

    nc = Bass()
    xin = nc.declare_dram_parameter("x", [NT, P, F], fp32, isOutput=False)
    yout = nc.declare_dram_parameter("y", [NT, P, F], int8, isOutput=True)

    with ExitStack() as ctx:
        block = ctx.enter_context(nc.Block())
        s_in = [ctx.enter_context(nc.semaphore(f"s_in{j}")) for j in range(DIN)]
        s_od = [ctx.enter_context(nc.semaphore(f"s_od{j}")) for j in range(DY)]
        s_p1 = ctx.enter_context(nc.semaphore("s_p1"))
        s_p2 = ctx.enter_context(nc.semaphore("s_p2"))
        xt = ctx.enter_context(nc.sbuf_tensor("xt", [P, DIN * F], fp32))
        tr = ctx.enter_context(nc.sbuf_tensor("tr", [P, DT * F], int16))
        ty = ctx.enter_context(nc.sbuf_tensor("ty", [P, DY * F], int8))

        def sl(t, j, d, w):
            return t[:, (j % d) * F:(j % d) * F + w]

        def dram(t, c):
            ti, lo, w = CHUNKS[c]
            return t[ti][:, lo:lo + w]

        @block.sync
        def _(sync):
            for i in range(NC_):
                if i >= DIN:
                    sync.wait_ge(s_p1, i - DIN + 1)   # p1 done reading xt slot
                sync.dma_start(
                    out=sl(xt, i, DIN, CHUNKS[i][2]), in_=dram(xin, i)
                ).then_inc(s_in[i % DIN], 16)

        @block.vector
        def _(vector):
            for ii in range(NC_ + 1):
                if ii < NC_:
                    vector.wait_ge(s_in[ii % DIN], 16 * (ii // DIN + 1))
                    if ii >= DT:
                        vector.wait_ge(s_p2, ii - DT + 1)  # p2 done with tr slot
                    vector.tensor_scalar(
                        out=sl(tr, ii, DT, CHUNKS[ii][2]),
                        in0=sl(xt, ii, DIN, CHUNKS[ii][2]),
                        scalar1=32.0, scalar2=0.5,
                        op0=alu.mult, op1=alu.add,
                    ).then_inc(s_p1, 1)
                if ii >= 1:
                    k = ii - 1
                    vector.wait_ge(s_p1, k + 1)           # own p1(k) committed
                    if k >= DY:
                        vector.wait_ge(s_od[k % DY], 16 * (k // DY))
                    vector.tensor_scalar(
                        out=sl(ty, k, DY, CHUNKS[k][2]),
                        in0=sl(tr, k, DT, CHUNKS[k][2]),
                        scalar1=0.5, scalar2=-0.25,
                        op0=alu.mult, op1=alu.add,
                    ).then_inc(s_p2, 1)

        @block.scalar
        def _(scalar):
            for k in range(NC_):
                scalar.wait_ge(s_p2, k + 1)
                scalar.dma_start(
                    out=dram(yout, k), in_=sl(ty, k, DY, CHUNKS[k][2])
                ).then_inc(s_od[k % DY], 16)

    return nc


def kernel(x: np.ndarray) -> np.ndarray:
    from concourse.bass_utils import run_bass_kernel_spmd

    if "nc" not in _cache:
        _cache["nc"] = _build()
    nc = _cache["nc"]

    xs = np.ascontiguousarray(x, dtype=np.float32).reshape(N_CORES, NT, P, F)
    in_maps = [{"x": xs[c]} for c in range(N_CORES)]

    trace = bool(os.environ.get("BASS_TRACE"))
    tmpdir = None
    base = os.environ.get("BASS_TRACE_DIR")
    if base:
        # unique dir per invocation: a reused dir trips gauge's
        # "parallel instances of neuron-profile on the same json path"
        import tempfile

        os.makedirs(base, exist_ok=True)
        tmpdir = tempfile.mkdtemp(prefix="call_", dir=base)
    res = run_bass_kernel_spmd(
        nc, in_maps, list(range(N_CORES)), trace=trace, tmpdir=tmpdir
    )
    if res.exec_time_ns is not None:
        print(f"HW exec time: {res.exec_time_ns} ns")

    out = np.concatenate(
        [np.asarray(res.results[c]["y"]).reshape(-1) for c in range(N_CORES)]
    )
    # k * 2^-4, exact (int8 saturation on device == the reference clamp)
    out = out.astype(np.float32)
    out *= np.float32(0.0625)
    return out.reshape(B, C, H, W)


# revision 18
# speedup vs baseline: 1.0276x; 1.0276x over previous
"""LinearQuant kernel for Trainium2 (8 NeuronCores, data parallel).

Reference math (fp32, bit-exact):
    delta = 2^-4; bound = 128
    out = clip(floor(x/delta + 0.5), -128, 127) * delta

Device pipeline (2 DVE tensor_scalar ops per tile, int8 output):

  r = RNE_int16(32*x + 0.5)          # TS mult+add, fp32 -> int16 (HW convert
                                     # is round-nearest-even + saturate;
                                     # |r| <= ~260 for any sane x)
  k = RNE_int8(r*0.5 - 0.25)         # TS mult+add, int16 -> int8: r even ->
                                     # k-0.25, r odd -> k+0.25, both RNE to
                                     # k = floor(16x+0.5) exactly (incl. ties,
                                     # negatives); int8 saturation == the
                                     # reference clamp to [-128, 127]

using the identity floor(u) = RNE(2u - 0.5) >> 1 with u = 16x + 0.5
(2u - 0.5 = 32x + 0.5), where the ">> 1" is done in float as *0.5 - 0.25
before the int8 convert. Verified bit-exact on HW for tie points, clamp
range, and 16K random normals (probe2.py). Host multiplies the int8 k by
2^-4 -- exact.

Storing int8 instead of bf16 halves store traffic: 25.7 MB in + 6.4 MB out
= 32.1 MB per core vs 38.5 MB for the bf16 baseline. Measured wire rate
while DMA-busy is ~410-415 GB/s (uncontended; the fabric ceiling is 435)
or ~355 GB/s when the paired NeuronCore / neighbors load the same HBM
stack, with zero mid-stream DMA gaps either way, so the kernel is at the
memory roofline: ~8.7 us fixed NEFF preamble + 32.1 MB wire time + ~3 us
tail latency (last in-DMA receipt -> p1 -> p2 -> out-DMA -> receipt ->
postamble). Measured 89.5-90.4 us uncontended, ~102-106 us contended
(baseline: 129-130 us).

Engine split: SP(sync) issues in-DMAs (HWDGE ring 1), DVE runs the two TS
passes software-pipelined (p1(i) || p2(i-1)) over ring buffers so no
same-engine commit stalls, ACT issues out-DMAs (HWDGE ring 2, no
activation => no ACT table load). Raw Block style with explicit
semaphores (Tile auto-sems hit walrus "Too many sync wait commands" on
this shape). Ring depths matter: DY=3 throttles out-DMA issue just
enough that write bursts stay coarse relative to the read stream --
DY=6 dropped the mixed read/write rate to ~343 GB/s (fine-grained HBM
read/write turnaround), DY=2 stalled p2 on out receipts, and issuing
everything on one HWDGE ring serialized in/out transfers (123 us).

Sharding: x(64,256,56,56) split 8-way along batch -> 6,422,528 elems/core
= 14 tiles of [128, 3584] fp32. The last tile is split into 4 quarter
chunks of [128, 896] so the post-last-in-DMA serial chain (p1+p2+out-DMA
+ DVE drain) shrinks from ~7 us to ~2 us.
"""

import os

import numpy as np

B, C, H, W = 64, 256, 56, 56
N_CORES = 8
P = 128          # partitions
F = 3584         # free elems per full tile
NT = 14          # full tiles per core:  8*256*56*56 == NT*P*F
DIN = 4          # input ring slots
DT = 3           # intermediate ring slots
DY = 3           # output ring slots
# chunk list: (tile_idx, col_lo, width); last tile tapered into quarters
CHUNKS = [(i, 0, F) for i in range(NT - 1)] + [
    (NT - 1, q * (F // 4), F // 4) for q in range(4)
]
NC_ = len(CHUNKS)

_cache = {}


def _build():
    from contextlib import ExitStack

    import concourse.mybir as mybir
    from concourse.bass import Bass

    fp32 = mybir.dt.float32
    int16 = mybir.dt.int16
    int8 = mybir.dt.int8
    alu = mybir.AluOpType

    nc = Bass()
    xin = nc.declare_dram_parameter("x", [NT, P, F], fp32, isOutput=False)
    yout = nc.declare_dram_parameter("y", [NT, P, F], int8, isOutput=True)

    with ExitStack() as ctx:
        block = ctx.enter_context(nc.Block())
        s_in = [ctx.enter_context(nc.semaphore(f"s_in{j}")) for j in range(DIN)]
        s_od = [ctx.enter_context(nc.semaphore(f"s_od{j}")) for j in range(DY)]
        s_odq = ctx.enter_context(nc.semaphore("s_odq"))
        s_p1 = ctx.enter_context(nc.semaphore("s_p1"))
        s_p2 = ctx.enter_context(nc.semaphore("s_p2"))
        xt = ctx.enter_context(nc.sbuf_tensor("xt", [P, DIN * F], fp32))
        tr = ctx.enter_context(nc.sbuf_tensor("tr", [P, DT * F], int16))
        ty = ctx.enter_context(nc.sbuf_tensor("ty", [P, DY * F], int8))
        # dedicated out buffer for the 4 tail quarters: their p2 never waits
        # on an out-DMA receipt, so the tail chain stays short even though
        # the last two full-tile outs are deferred (see scalar block)
        tyq = ctx.enter_context(nc.sbuf_tensor("tyq", [P, F], int8))
        NQ = NC_ - 4                      # first quarter chunk index (13)
        # s_in[] value when the final in-DMA (chunk NC_-1) has completed
        LAST_IN_SLOT = (NC_ - 1) % DIN
        LAST_IN_VAL = 16 * ((NC_ - 1) // DIN + 1)

        def sl(t, j, d, w):
            return t[:, (j % d) * F:(j % d) * F + w]

        def dram(t, c):
            ti, lo, w = CHUNKS[c]
            return t[ti][:, lo:lo + w]

        @block.sync
        def _(sync):
            for i in range(NC_):
                if i >= DIN:
                    sync.wait_ge(s_p1, i - DIN + 1)   # p1 done reading xt slot
                sync.dma_start(
                    out=sl(xt, i, DIN, CHUNKS[i][2]), in_=dram(xin, i)
                ).then_inc(s_in[i % DIN], 16)

        @block.vector
        def _(vector):
            for ii in range(NC_ + 1):
                if ii < NC_:
                    vector.wait_ge(s_in[ii % DIN], 16 * (ii // DIN + 1))
                    if ii >= DT:
                        vector.wait_ge(s_p2, ii - DT + 1)  # p2 done with tr slot
                    vector.tensor_scalar(
                        out=sl(tr, ii, DT, CHUNKS[ii][2]),
                        in0=sl(xt, ii, DIN, CHUNKS[ii][2]),
                        scalar1=32.0, scalar2=0.5,
                        op0=alu.mult, op1=alu.add,
                    ).then_inc(s_p1, 1)
                if ii >= 1:
                    k = ii - 1
                    vector.wait_ge(s_p1, k + 1)           # own p1(k) committed
                    if k >= NQ:
                        yslice = tyq[:, (k - NQ) * (F // 4):
                                     (k - NQ + 1) * (F // 4)]
                    else:
                        if k >= DY:
                            vector.wait_ge(s_od[k % DY], 16 * (k // DY))
                        yslice = sl(ty, k, DY, CHUNKS[k][2])
                    vector.tensor_scalar(
                        out=yslice,
                        in0=sl(tr, k, DT, CHUNKS[k][2]),
                        scalar1=0.5, scalar2=-0.25,
                        op0=alu.mult, op1=alu.add,
                    ).then_inc(s_p2, 1)

        @block.scalar
        def _(scalar):
            for k in range(NC_):
                if k >= NQ - 2:
                    # defer the last two full-tile outs (and the quarter
                    # outs) until the final in-DMA has landed: their
                    # packets would otherwise compete with the last
                    # in-DMAs and push the tail chain ~2 us later
                    scalar.wait_ge(s_in[LAST_IN_SLOT], LAST_IN_VAL)
                scalar.wait_ge(s_p2, k + 1)
                if k >= NQ:
                    scalar.dma_start(
                        out=dram(yout, k),
                        in_=tyq[:, (k - NQ) * (F // 4):
                                (k - NQ + 1) * (F // 4)],
                    ).then_inc(s_odq, 16)
                else:
                    scalar.dma_start(
                        out=dram(yout, k), in_=sl(ty, k, DY, CHUNKS[k][2])
                    ).then_inc(s_od[k % DY], 16)

    return nc


def kernel(x: np.ndarray) -> np.ndarray:
    from concourse.bass_utils import run_bass_kernel_spmd

    if "nc" not in _cache:
        _cache["nc"] = _build()
    nc = _cache["nc"]

    xs = np.ascontiguousarray(x, dtype=np.float32).reshape(N_CORES, NT, P, F)
    in_maps = [{"x": xs[c]} for c in range(N_CORES)]

    trace = bool(os.environ.get("BASS_TRACE"))
    tmpdir = None
    base = os.environ.get("BASS_TRACE_DIR")
    if base:
        # unique dir per invocation: a reused dir trips gauge's
        # "parallel instances of neuron-profile on the same json path"
        import tempfile

        os.makedirs(base, exist_ok=True)
        tmpdir = tempfile.mkdtemp(prefix="call_", dir=base)
    res = run_bass_kernel_spmd(
        nc, in_maps, list(range(N_CORES)), trace=trace, tmpdir=tmpdir
    )
    if res.exec_time_ns is not None:
        print(f"HW exec time: {res.exec_time_ns} ns")

    out = np.concatenate(
        [np.asarray(res.results[c]["y"]).reshape(-1) for c in range(N_CORES)]
    )
    # k * 2^-4, exact (int8 saturation on device == the reference clamp)
    out = out.astype(np.float32)
    out *= np.float32(0.0625)
    return out.reshape(B, C, H, W)


# revision 19
# speedup vs baseline: 1.1891x; 1.1572x over previous
"""LinearQuant kernel for Trainium2 (8 NeuronCores, data parallel).

Reference math (fp32, bit-exact):
    delta = 2^-4; bound = 128
    out = clip(floor(x/delta + 0.5), -128, 127) * delta

Device pipeline (2 DVE tensor_scalar ops per tile, int8 output):

  r = RNE_int16(32*x + 0.5)          # TS mult+add, fp32 -> int16 (HW convert
                                     # is round-nearest-even + saturate;
                                     # |r| <= ~260 for any sane x)
  k = RNE_int8(r*0.5 - 0.25)         # TS mult+add, int16 -> int8: r even ->
                                     # k-0.25, r odd -> k+0.25, both RNE to
                                     # k = floor(16x+0.5) exactly (incl. ties,
                                     # negatives); int8 saturation == the
                                     # reference clamp to [-128, 127]

using the identity floor(u) = RNE(2u - 0.5) >> 1 with u = 16x + 0.5
(2u - 0.5 = 32x + 0.5), where the ">> 1" is done in float as *0.5 - 0.25
before the int8 convert. Verified bit-exact on HW for tie points, clamp
range, and 16K random normals (probe2.py). Host multiplies the int8 k by
2^-4 -- exact.

Storing int8 instead of bf16 halves store traffic: 25.7 MB in + 6.4 MB out
= 32.1 MB per core vs 38.5 MB for the bf16 baseline. Measured wire rate
while DMA-busy is ~410-415 GB/s (uncontended; the fabric ceiling is 435)
or ~355 GB/s when the paired NeuronCore / neighbors load the same HBM
stack, with zero mid-stream DMA gaps either way, so the kernel is at the
memory roofline: ~8.7 us fixed NEFF preamble + 32.1 MB wire time + ~3 us
tail latency (last in-DMA receipt -> p1 -> p2 -> out-DMA -> receipt ->
postamble). Measured 89.5-90.4 us uncontended, ~102-106 us contended
(baseline: 129-130 us).

Engine split: SP(sync) issues in-DMAs (HWDGE ring 1), DVE runs the two TS
passes software-pipelined (p1(i) || p2(i-1)) over ring buffers so no
same-engine commit stalls, ACT issues out-DMAs (HWDGE ring 2, no
activation => no ACT table load). Raw Block style with explicit
semaphores (Tile auto-sems hit walrus "Too many sync wait commands" on
this shape). Ring depths matter: DY=3 throttles out-DMA issue just
enough that write bursts stay coarse relative to the read stream --
DY=6 dropped the mixed read/write rate to ~343 GB/s (fine-grained HBM
read/write turnaround), DY=2 stalled p2 on out receipts, and issuing
everything on one HWDGE ring serialized in/out transfers (123 us).

Sharding: x(64,256,56,56) split 8-way along batch -> 6,422,528 elems/core
= 14 tiles of [128, 3584] fp32. The last tile is split into 4 quarter
chunks of [128, 896] so the post-last-in-DMA serial chain (p1+p2+out-DMA
+ DVE drain) shrinks from ~7 us to ~2 us.
"""

import os

import numpy as np

B, C, H, W = 64, 256, 56, 56
N_CORES = 8
P = 128          # partitions
F = 3584         # free elems per full tile
NT = 14          # full tiles per core:  8*256*56*56 == NT*P*F
DIN = 4          # input ring slots
DT = 3           # intermediate ring slots
DY = 3           # output ring slots
# chunk list: (tile_idx, col_lo, width); last tile tapered into quarters
CHUNKS = [(i, 0, F) for i in range(NT - 1)] + [
    (NT - 1, q * (F // 4), F // 4) for q in range(4)
]
NC_ = len(CHUNKS)

_cache = {}


def _build():
    from contextlib import ExitStack

    import concourse.mybir as mybir
    from concourse.bass import Bass

    fp32 = mybir.dt.float32
    int16 = mybir.dt.int16
    int8 = mybir.dt.int8
    alu = mybir.AluOpType

    nc = Bass()
    xin = nc.declare_dram_parameter("x", [NT, P, F], fp32, isOutput=False)
    yout = nc.declare_dram_parameter("y", [NT, P, F], int8, isOutput=True)

    with ExitStack() as ctx:
        block = ctx.enter_context(nc.Block())
        s_in = [ctx.enter_context(nc.semaphore(f"s_in{j}")) for j in range(DIN)]
        s_od = [ctx.enter_context(nc.semaphore(f"s_od{j}")) for j in range(DY)]
        s_odq = ctx.enter_context(nc.semaphore("s_odq"))
        s_p1 = ctx.enter_context(nc.semaphore("s_p1"))
        s_p2 = ctx.enter_context(nc.semaphore("s_p2"))
        xt = ctx.enter_context(nc.sbuf_tensor("xt", [P, DIN * F], fp32))
        tr = ctx.enter_context(nc.sbuf_tensor("tr", [P, DT * F], int16))
        ty = ctx.enter_context(nc.sbuf_tensor("ty", [P, DY * F], int8))
        # dedicated out buffer for the 4 tail quarters: their p2 never waits
        # on an out-DMA receipt, so the tail chain stays short even though
        # the last two full-tile outs are deferred (see scalar block)
        tyq = ctx.enter_context(nc.sbuf_tensor("tyq", [P, F], int8))
        NQ = NC_ - 4                      # first quarter chunk index (13)
        # s_in[] value when the final in-DMA (chunk NC_-1) has completed
        LAST_IN_SLOT = (NC_ - 1) % DIN
        LAST_IN_VAL = 16 * ((NC_ - 1) // DIN + 1)

        def sl(t, j, d, w):
            return t[:, (j % d) * F:(j % d) * F + w]

        def dram(t, c):
            ti, lo, w = CHUNKS[c]
            return t[ti][:, lo:lo + w]

        @block.sync
        def _(sync):
            for i in range(NC_):
                if i >= DIN:
                    sync.wait_ge(s_p1, i - DIN + 1)   # p1 done reading xt slot
                sync.dma_start(
                    out=sl(xt, i, DIN, CHUNKS[i][2]), in_=dram(xin, i)
                ).then_inc(s_in[i % DIN], 16)

        @block.vector
        def _(vector):
            for ii in range(NC_ + 1):
                if ii < NC_:
                    vector.wait_ge(s_in[ii % DIN], 16 * (ii // DIN + 1))
                    if ii >= DT:
                        vector.wait_ge(s_p2, ii - DT + 1)  # p2 done with tr slot
                    vector.tensor_scalar(
                        out=sl(tr, ii, DT, CHUNKS[ii][2]),
                        in0=sl(xt, ii, DIN, CHUNKS[ii][2]),
                        scalar1=32.0, scalar2=0.5,
                        op0=alu.mult, op1=alu.add,
                    ).then_inc(s_p1, 1)
                if ii >= 1:
                    k = ii - 1
                    vector.wait_ge(s_p1, k + 1)           # own p1(k) committed
                    if k >= NQ:
                        yslice = tyq[:, (k - NQ) * (F // 4):
                                     (k - NQ + 1) * (F // 4)]
                    else:
                        if k >= DY:
                            vector.wait_ge(s_od[k % DY], 16 * (k // DY))
                        yslice = sl(ty, k, DY, CHUNKS[k][2])
                    vector.tensor_scalar(
                        out=yslice,
                        in0=sl(tr, k, DT, CHUNKS[k][2]),
                        scalar1=0.5, scalar2=-0.25,
                        op0=alu.mult, op1=alu.add,
                    ).then_inc(s_p2, 1)

        @block.scalar
        def _(scalar):
            for k in range(NC_):
                if k >= NQ - 1:
                    # defer the last full-tile out and the quarter outs
                    # until the final in-DMA has landed: their packets
                    # would otherwise compete with the last in-DMAs and
                    # push the tail chain later; the deferred ~0.9 MB
                    # flushes under the tail compute chain
                    scalar.wait_ge(s_in[LAST_IN_SLOT], LAST_IN_VAL)
                scalar.wait_ge(s_p2, k + 1)
                if k >= NQ:
                    scalar.dma_start(
                        out=dram(yout, k),
                        in_=tyq[:, (k - NQ) * (F // 4):
                                (k - NQ + 1) * (F // 4)],
                    ).then_inc(s_odq, 16)
                else:
                    scalar.dma_start(
                        out=dram(yout, k), in_=sl(ty, k, DY, CHUNKS[k][2])
                    ).then_inc(s_od[k % DY], 16)

    return nc


def kernel(x: np.ndarray) -> np.ndarray:
    from concourse.bass_utils import run_bass_kernel_spmd

    if "nc" not in _cache:
        _cache["nc"] = _build()
    nc = _cache["nc"]

    xs = np.ascontiguousarray(x, dtype=np.float32).reshape(N_CORES, NT, P, F)
    in_maps = [{"x": xs[c]} for c in range(N_CORES)]

    trace = bool(os.environ.get("BASS_TRACE"))
    tmpdir = None
    base = os.environ.get("BASS_TRACE_DIR")
    if base:
        # unique dir per invocation: a reused dir trips gauge's
        # "parallel instances of neuron-profile on the same json path"
        import tempfile

        os.makedirs(base, exist_ok=True)
        tmpdir = tempfile.mkdtemp(prefix="call_", dir=base)
    res = run_bass_kernel_spmd(
        nc, in_maps, list(range(N_CORES)), trace=trace, tmpdir=tmpdir
    )
    if res.exec_time_ns is not None:
        print(f"HW exec time: {res.exec_time_ns} ns")

    out = np.concatenate(
        [np.asarray(res.results[c]["y"]).reshape(-1) for c in range(N_CORES)]
    )
    # k * 2^-4, exact (int8 saturation on device == the reference clamp)
    out = out.astype(np.float32)
    out *= np.float32(0.0625)
    return out.reshape(B, C, H, W)
